# revision 1
# baseline (speedup 1.0000x reference)
"""Segment-mean aggregation kernel for Trainium2 (8 NeuronCores).

Problem: out[b, m, :] = mean over edges e with dst[e]==m of grid[b, src[e], :]
  grid: [4, 262144, 64] f32, edge_index: [1048576, 2] (src, dst) in [0, 40962).

Only grid rows < 40962 are ever referenced (src values are mesh-node ids), so
the kernel gathers from a batch-packed table [40962, 4*64] f32 (1024 B rows).

Device algorithm (per core, SPMD over 8 cores):
  - dst space is cut into 128-row chunks; chunks are sorted by edge count and
    dealt round-robin to (core, slot) so per-slot capacities are tight across
    cores.  Per chunk the core issues two dma_gather ops (int16 indices only
    address 32768 rows, so the table is addressed in two halves), landing
    gathered rows in SBUF as [128 edges, 256 feat] tiles.
  - Per 128-edge tile a one-hot matrix W[e, d] = (dst_local[e] == d) is built
    with one DVE tensor_scalar(is_equal) against a static iota row, and
    PSUM[d, f] += W.T @ G accumulates the scatter-sum on the PE (float32r).
  - The PSUM chunk is copied to SBUF and DMA'd to the per-core output slab.
Host does the cheap O(E) index prep and the final divide-by-counts.
"""

import numpy as np
from dataclasses import dataclass

P = 128  # partitions / chunk width


def r16(x):
    return (int(x) + 15) & ~15


@dataclass(frozen=True)
class Cfg:
    n_src: int            # rows in gather table
    n_dst: int            # dst (mesh) rows
    feat: int             # packed feature width (B * D)
    n_cores: int
    half: int             # int16 index window split of the src table
    cap0: tuple           # per-slot edge capacity (multiple of 16), src < half
    cap1: tuple           # per-slot edge capacity, src >= half
    precision: str = "bf16x2"   # "f32r" | "bf16x2" (hi/lo bf16 split)
    repeat: int = 1             # run the main loop N times (timing calib)
    ablate: str = ""            # "" | "nogather" | "nocompute" (perf debug)
    gsplit: int = 1024          # max idxs per dma_gather (1024 HW-verified)

    @property
    def n_chunks(self):
        return -(-self.n_dst // P)

    @property
    def n_slots(self):
        return len(self.cap0)

    def tiles0(self, j):
        return -(-self.cap0[j] // P)

    def tiles1(self, j):
        return -(-self.cap1[j] // P)

    def ntiles(self, j):
        return self.tiles0(j) + self.tiles1(j)

    @property
    def max_tiles(self):
        return max(self.ntiles(j) for j in range(self.n_slots))


def plan(src, dst, n_src, n_dst, n_cores):
    """Order chunks by size, deal round-robin, derive per-slot capacities.

    Returns (cfg, chunk_order) where chunk_order[r] is the dst chunk handled
    by core r % n_cores, slot r // n_cores."""
    half = 32768
    n_chunks = -(-n_dst // P)
    n_slots = -(-n_chunks // n_cores)
    chunk = dst // P
    h = src >= half
    g0 = np.bincount(chunk[~h], minlength=n_chunks)
    g1 = np.bincount(chunk[h], minlength=n_chunks)
    chunk_order = np.argsort(-(g0 + g1), kind="stable")

    cap0, cap1 = [], []
    for j in range(n_slots):
        sel = chunk_order[j * n_cores:(j + 1) * n_cores]
        cap0.append(max(r16(g0[sel].max(initial=0)), 16))
        cap1.append(max(r16(g1[sel].max(initial=0)), 16))
    cfg = Cfg(n_src=n_src, n_dst=n_dst, feat=0, n_cores=n_cores, half=half,
              cap0=tuple(cap0), cap1=tuple(cap1))
    return cfg, chunk_order


def pack_table(cfg, table_f32):
    """f32 [n, F] -> gather payload: f32 as-is, or packed [hi|lo] bf16
    [n, 2F] for the bf16x2 precision mode."""
    if cfg.precision != "bf16x2":
        return table_f32
    import ml_dtypes
    hi = table_f32.astype(ml_dtypes.bfloat16)
    lo = (table_f32 - hi.astype(np.float32)).astype(ml_dtypes.bfloat16)
    return np.ascontiguousarray(np.concatenate([hi, lo], axis=1))


def prep(cfg, chunk_order, table, src, dst):
    """Build per-core input maps.  table: [n_src, feat] f32 contiguous."""
    C, S = cfg.n_cores, cfg.n_slots
    E = src.shape[0]

    # rank of each chunk in the dealt order
    chunk_rank = np.empty(cfg.n_chunks, np.int64)
    chunk_rank[chunk_order] = np.arange(cfg.n_chunks)

    chunk = dst // P
    rank = chunk_rank[chunk]
    core = rank % C
    slot = rank // C
    h = (src >= cfg.half).astype(np.int64)

    key = (core * S + slot) * 2 + h
    order = np.argsort(key, kind="stable")
    skey = key[order]
    gcnt = np.bincount(key, minlength=C * S * 2)
    gstart = np.concatenate([[0], np.cumsum(gcnt)])[:-1]
    pos = np.arange(E) - gstart[skey]

    # per-slot layout offsets
    cap0 = np.array(cfg.cap0)
    cap1 = np.array(cfg.cap1)
    gcap = np.array([cfg.ntiles(j) * P for j in range(S)])
    slot_base = np.concatenate([[0], np.cumsum(gcap)])  # edge-position space
    tot_e = int(slot_base[-1])
    t0 = np.array([cfg.tiles0(j) for j in range(S)])

    for j in range(S):
        m0 = gcnt.reshape(C, S, 2)[:, j, 0].max()
        m1 = gcnt.reshape(C, S, 2)[:, j, 1].max()
        assert m0 <= cap0[j] and m1 <= cap1[j], (j, m0, cap0[j], m1, cap1[j])

    sslot = (skey // 2) % S
    sh = skey % 2
    score = skey // (2 * S)
    padpos = slot_base[sslot] + sh * t0[sslot] * P + pos

    srcidx = np.zeros((C, tot_e), np.int16)
    dstsel = np.full((C, tot_e), 255.0, np.float32)
    srcidx[score, padpos] = (src[order] - sh * cfg.half).astype(np.int16)
    dstsel[score, padpos] = (dst[order] - chunk[order] * P).astype(np.float32)

    # int16 index SBUF layout: per (slot, half) group the indices are laid out
    # i -> (partition i%16, col i//16), 16-row block replicated 8x to 128.
    # Group g's column window is [colo[g], colo[g] + cap/16).
    ncol0 = cap0 // 16
    ncol1 = cap1 // 16
    colo = np.concatenate([[0], np.cumsum(ncol0 + ncol1)])
    tot_cols = int(colo[-1])
    srcidx_sb = np.zeros((C, 16, tot_cols), np.int16)
    for j in range(S):
        b = slot_base[j]
        a0 = srcidx[:, b: b + cap0[j]].reshape(C, ncol0[j], 16)
        srcidx_sb[:, :, colo[j]: colo[j] + ncol0[j]] = a0.transpose(0, 2, 1)
        b1 = b + t0[j] * P
        a1 = srcidx[:, b1: b1 + cap1[j]].reshape(C, ncol1[j], 16)
        srcidx_sb[:, :, colo[j] + ncol0[j]: colo[j] + ncol0[j] + ncol1[j]] = \
            a1.transpose(0, 2, 1)
    srcidx_sb = np.tile(srcidx_sb, (1, 8, 1))

    # dstsel SBUF layout: [128, total_tiles]; tile t partition p = edge t*128+p
    dstsel_sb = dstsel.reshape(C, tot_e // P, P).transpose(0, 2, 1).copy()

    iota = np.tile(np.arange(P, dtype=np.float32), (P, 1))

    in_maps = [{"table": table, "srcidx": srcidx_sb[c], "dstsel": dstsel_sb[c],
                "iota": iota} for c in range(C)]
    aux = {"colo": colo, "ncol0": ncol0, "ncol1": ncol1,
           "chunk_order": chunk_order}
    return in_maps, aux


def build(cfg):
    import concourse.bacc as bacc
    import concourse.tile as tile
    from concourse import mybir

    f32 = mybir.dt.float32
    f32r = mybir.dt.float32r
    bf16 = mybir.dt.bfloat16
    i16 = mybir.dt.int16

    C, S, F = cfg.n_cores, cfg.n_slots, cfg.feat
    ncol0 = [cfg.cap0[j] // 16 for j in range(S)]
    ncol1 = [cfg.cap1[j] // 16 for j in range(S)]
    colo = np.concatenate([[0], np.cumsum(np.array(ncol0) + np.array(ncol1))])
    tot_cols = int(colo[-1])
    tot_tiles = sum(cfg.ntiles(j) for j in range(S))

    hilo = cfg.precision == "bf16x2"
    gdt = bf16 if hilo else f32r       # gathered-data dtype
    gF = 2 * F if hilo else F          # gathered row width in gdt elems

    nc = bacc.Bacc("TRN2", target_bir_lowering=False, debug=False)
    table = nc.dram_tensor("table", [cfg.n_src, gF], gdt,
                           kind="ExternalInput")
    srcidx = nc.dram_tensor("srcidx", [P, tot_cols], i16,
                            kind="ExternalInput")
    dstsel = nc.dram_tensor("dstsel", [P, tot_tiles], f32,
                            kind="ExternalInput")
    iota = nc.dram_tensor("iota", [P, P], f32, kind="ExternalInput")
    out = nc.dram_tensor("out", [S * P, F], f32, kind="ExternalOutput")

    with tile.TileContext(nc) as tc:
        with (
            tc.tile_pool(name="meta", bufs=1) as meta,
            tc.tile_pool(name="onehot", bufs=6) as wpool,
            tc.tile_pool(name="outsb", bufs=4) as opool,
            tc.tile_pool(name="psum", bufs=2, space="PSUM") as ppool,
        ):
            srcidx_sb = meta.tile([P, tot_cols], i16)
            dstsel_sb = meta.tile([P, tot_tiles], f32)
            iota_sb = meta.tile([P, P], f32)
            nc.sync.dma_start(srcidx_sb[:], srcidx[:])
            nc.sync.dma_start(dstsel_sb[:], dstsel[:])
            nc.sync.dma_start(iota_sb[:], iota[:])

            NBUF = 3
            NTMAX = cfg.max_tiles
            gball = meta.tile([P, NBUF * NTMAX * gF], gdt)
            nc.vector.memset(gball[:] if hilo else gball[:].bitcast(f32), 0.0)
            gb3 = gball[:].rearrange("p (s f) -> p s f", f=gF)

            tbase = 0
            for j2 in range(S * cfg.repeat):
                j = j2 % S
                if j == 0:
                    tbase = 0
                gbase = (j2 % NBUF) * NTMAX
                nt = cfg.ntiles(j)
                for h in range(2):
                    cap = (cfg.cap0[j], cfg.cap1[j])[h]
                    slotbase = gbase + (0 if h == 0 else cfg.tiles0(j))
                    colbase = int(colo[j]) + (0 if h == 0 else ncol0[j])
                    in_ap = table[: cfg.half, :] if h == 0 else \
                        table[cfg.half:, :]
                    # <=gsplit idxs per gather instruction (64 descriptors
                    # at 1024; >~2900 crashes the device)
                    for s in range(0, cap, cfg.gsplit):
                        if cfg.ablate == "nogather":
                            break
                        n = min(cfg.gsplit, cap - s)
                        sl0 = slotbase + s // P
                        nsl = -(-n // P)
                        nc.gpsimd.dma_gather(
                            out_ap=gb3[:, sl0: sl0 + nsl, :],
                            in_ap=in_ap,
                            idxs_ap=srcidx_sb[:, colbase + s // 16:
                                              colbase + s // 16 + n // 16],
                            num_idxs=n,
                            num_idxs_reg=n,
                            elem_size=gF,
                        )
                if cfg.ablate == "nocompute":
                    continue
                psum = ppool.tile([P, F], f32)
                for t in range(nt):
                    w = wpool.tile([P, P], gdt)
                    nc.vector.tensor_scalar(
                        out=w[:], in0=iota_sb[:],
                        scalar1=dstsel_sb[:, tbase + t: tbase + t + 1],
                        scalar2=None, op0=mybir.AluOpType.is_equal)
                    if hilo:
                        nc.tensor.matmul(
                            out=psum[:], lhsT=w[:],
                            rhs=gb3[:, gbase + t, 0:F],
                            start=(t == 0), stop=False)
                        nc.tensor.matmul(
                            out=psum[:], lhsT=w[:],
                            rhs=gb3[:, gbase + t, F:2 * F],
                            start=False, stop=(t == nt - 1))
                    else:
                        nc.tensor.matmul(
                            out=psum[:], lhsT=w[:],
                            rhs=gb3[:, gbase + t, :],
                            start=(t == 0), stop=(t == nt - 1))
                tbase += nt
                osb = opool.tile([P, F], f32)
                nc.vector.tensor_copy(out=osb[:], in_=psum[:])
                nc.sync.dma_start(out[j * P:(j + 1) * P, :], osb[:])
    nc.compile()
    return nc


def assemble(cfg, chunk_order, core_outs, counts):
    """core_outs: list of [S*128, feat] per-core slabs -> [n_dst, feat] mean"""
    C, S = cfg.n_cores, cfg.n_slots
    stacked = np.stack([o.reshape(S, P, cfg.feat) for o in core_outs])
    r = np.arange(cfg.n_chunks)
    full = np.zeros((S * C * P, cfg.feat), np.float32)
    full.reshape(S * C, P, cfg.feat)[chunk_order] = stacked[r % C, r // C]
    full = full[: cfg.n_dst]
    return full / np.maximum(counts, 1.0)[:, None]


_CACHE = {}
LAST_RESULT = None  # BassKernelResults of the most recent run (for profiling)


def kernel(grid_node_features, edge_index):
    grid = np.asarray(grid_node_features, dtype=np.float32)
    edges = np.asarray(edge_index)
    B, _, D = grid.shape
    NM = 40962
    src = edges[:, 0].astype(np.int64)
    dst = edges[:, 1].astype(np.int64)

    cfg, chunk_order = plan(src, dst, n_src=NM, n_dst=NM, n_cores=8)
    cfg = Cfg(**{**cfg.__dict__, "feat": B * D})
    table = np.ascontiguousarray(
        grid[:, :NM, :].transpose(1, 0, 2).reshape(NM, B * D))
    table = pack_table(cfg, table)
    in_maps, aux = prep(cfg, chunk_order, table, src, dst)
    counts = np.bincount(dst, minlength=NM).astype(np.float32)

    if cfg not in _CACHE:
        _CACHE[cfg] = build(cfg)
    nc = _CACHE[cfg]

    from concourse.bass_utils import run_bass_kernel_spmd
    res = run_bass_kernel_spmd(nc, in_maps, core_ids=list(range(cfg.n_cores)))
    global LAST_RESULT
    LAST_RESULT = res
    core_outs = [r["out"] for r in res.results]

    full = assemble(cfg, chunk_order, core_outs, counts)  # [NM, B*D]
    out = full.reshape(NM, B, D).transpose(1, 0, 2)       # [B, NM, D]
    return np.ascontiguousarray(out, dtype=np.float32)

